# revision 29
# baseline (speedup 1.0000x reference)
"""BatchHardTripletLoss on 8 Trainium2 NeuronCores.

Strategy (row-parallel, per the sharding hint):
  - Host: sort rows by label (loss is a mean over anchors -- any permutation
    is loss-invariant), L2-normalize in f32, and hand each core a ROTATED
    copy of the normalized feature matrix, transposed and cast to fp8 e4m3:
    core c's candidate column j holds sorted row (j + 1024c - 512) mod 8192.
    After rotation every core's 1024 anchors sit at fixed columns
    [512, 1536), and each anchor tile m's positive candidates lie inside a
    fixed 1024-wide window at columns [512*WMAP[m], 512*WMAP[m]+1024).
    All per-core differences are data (rotated ftr, window mask), never
    code, so one NEFF runs SPMD on all 8 cores.  (fp8 quantization of the
    normalized features moves the final mean loss by ~7e-4 relative --
    measured, far inside the 2e-2 gate -- while doubling PE throughput.)
  - Math: with normalized features, d2_ij = 2 - 2*g_ij and sqrt/affine are
    monotone, so only per-row min/max of the gram matrix are needed.
    hardest_neg gram = max over all candidates of (g - 2*pos_mask)
    hardest_pos gram = min over window of (g - 2*pos_mask) + 2
    (the -2 penalty is fused into the PE accumulation as an extra matmul
    of a constant -2*I against the fp8 mask -- no NxN elementwise pass;
    g >= -1 for unit vectors, so any penalized positive sits below every
    negative, and the shift keeps values in [-3,1] where bf16 is fine).
  - Device per core: 64 gram chunks (8 anchor tiles x 8 column chunks of
    1024); the 256-deep contraction runs as one fp8 DoubleRow matmul per
    512 output cols.  The two window chunks of each anchor tile get the
    mask fused in PSUM.  PSUM (4 chunk slots) is drained by two engines
    in parallel: ScalarE evicts a chunk to SBUF bf16 and DVE reduces it
    in 4x mode (window chunks also yield the hardest-pos half-mins
    there), or DVE reduces directly from PSUM at 1x; the chunk order and
    evict/direct split are chosen greedily at build time so both drain
    engines stay busy.  Tiny f32 epilogue (sqrt via ACT with the
    d2 = -2g+2 affine fused in, relu, row-sum, partition-sum via matmul
    with ones) produces one scalar per core; host averages.
"""

import numpy as np
import ml_dtypes

N = 8192
D = 256
NCORES = 8
CA = N // NCORES          # anchors per core
MT = CA // 128            # 8 anchor tiles per core
NCH = N // 1024           # 8 candidate DMA chunks of 1024
WMAP = [0, 1, 1, 1, 1, 2, 2, 2]   # window start half (of 512) per anchor tile
BIG = 2.0
AOFF = 512                # anchors occupy rotated cols [512, 1536)

_CACHE = {}


def _build(reps=1):
    import concourse.bass as bass
    import concourse.tile as tile
    from concourse import bacc, mybir, masks
    from contextlib import ExitStack

    F32 = mybir.dt.float32
    BF16 = mybir.dt.bfloat16
    FP8 = mybir.dt.float8e4
    DR = mybir.MatmulPerfMode.DoubleRow
    OP = mybir.AluOpType
    AX = mybir.AxisListType

    nc = bacc.Bacc("TRN2", target_bir_lowering=False, debug=False,
                   num_devices=NCORES)

    ftr_d = nc.dram_tensor("ftr", [128, NCH, 2, 1024], FP8,
                           kind="ExternalInput").ap()
    wm_d = nc.dram_tensor("wm", [128, MT, 1024], FP8,
                          kind="ExternalInput").ap()
    out_d = nc.dram_tensor("out", [128, 1], F32,
                           kind="ExternalOutput").ap()

    with tile.TileContext(nc) as tc:
      with ExitStack() as octx:
        # Constants once; the input tiles double-buffer across reps so
        # rep i+1's DMAs overlap rep i's compute instead of WAR-stalling
        # on the same SBUF addresses.
        constp = octx.enter_context(tc.tile_pool(name="const", bufs=1))
        ident = constp.tile([128, 128], F32)
        masks.make_identity(nc, ident[:])
        # First ACT op is a sqrt so bass loads a table set that holds
        # both sqrt and copy -- avoids a mid-kernel table switch.
        warm = constp.tile([1, 1], F32)
        nc.gpsimd.memset(warm[:], 1.0)
        nc.scalar.sqrt(warm[:], warm[:])
        negid = constp.tile([128, 128], FP8)
        nc.scalar.mul(negid[:], ident[:], -BIG)
        bias2 = constp.tile([128, 1], F32)
        nc.gpsimd.memset(bias2[:], 2.0)
        bigp = octx.enter_context(tc.tile_pool(name="big", bufs=2))

        for _rep in range(reps):
          with ExitStack() as ctx:
            ftr = bigp.tile([128, NCH, 2, 1024], FP8, tag="ftr")
            wm = bigp.tile([128, MT, 1024], FP8, tag="wm")

            # Two DMA queues (SP + ACT) in parallel; anchor/first-sweep
            # chunks 0-3 land first, mask before the p=0 sweep needs it.
            nc.sync.dma_start(ftr[:, 0], ftr_d[:, 0])
            nc.scalar.dma_start(ftr[:, 1], ftr_d[:, 1])
            nc.sync.dma_start(ftr[:, 2], ftr_d[:, 2])
            nc.scalar.dma_start(ftr[:, 3], ftr_d[:, 3])
            nc.sync.dma_start(ftr[:, 4], ftr_d[:, 4])
            nc.scalar.dma_start(ftr[:, 5], ftr_d[:, 5])
            nc.sync.dma_start(ftr[:, 6], ftr_d[:, 6])
            nc.scalar.dma_start(ftr[:, 7], ftr_d[:, 7])
            # mask last: the window chunks run late in the chunk order
            nc.sync.dma_start(wm[:], wm_d[:])

            vecp = ctx.enter_context(tc.tile_pool(name="vec", bufs=1))
            bpmax = vecp.tile([128, MT, NCH], F32)
            pmin = vecp.tile([128, MT, 2], F32)

            with ExitStack() as bctx:
                psB = bctx.enter_context(
                    tc.tile_pool(name="psB", bufs=4, space="PSUM"))
                sbB = bctx.enter_context(tc.tile_pool(name="sbB", bufs=6))

                # Chunk visit order + drain-engine assignment: the p=1
                # sweep warms up on chunks 0-3 with strict evict/direct
                # alternation; the rest (window chunks, which are always
                # evicted and also yield the bf16 hardest-pos mins, plus
                # the remaining plain chunks) are ordered greedily so the
                # running ScalarE and DVE drain totals stay balanced.
                def build_order():
                    mins_of = lambda cc, m: sum(
                        1 for h in (WMAP[m], WMAP[m] + 1) if h // 2 == cc)
                    order = []
                    for i, m in enumerate((0, 3, 2, 5, 4, 7, 6, 1)):
                        order.append((2, m, i % 2 == 0))
                        order.append((3, m, i % 2 == 1))
                    rest = ([(cc, m, True) for cc in (0, 1)
                             for m in range(MT)]
                            + [(cc, m, True) for cc in (4, 6)
                               for m in range(MT)]
                            + [(cc, m, True) for cc in (5,)
                               for m in (0,)]
                            + [(cc, m, False) for cc in (5,)
                               for m in (1, 2, 3, 4, 5, 6, 7)]
                            + [(cc, m, False) for cc in (7,)
                               for m in range(MT)])
                    acc = {"A": 0.0, "D": 0.0}
                    pool = list(rest)
                    while pool:
                        # pick the candidate that keeps |A - D| smallest;
                        # window chunks wait until slot 12 (mask DMA).
                        best = None
                        for it in pool:
                            cc, m, ev = it
                            if cc < 2 and len(order) < 28:
                                continue
                            a, d = acc["A"], acc["D"]
                            if ev:
                                a += 1038
                                d += 327 + 194 * mins_of(cc, m)
                            else:
                                d += 1127
                            sc = abs(a - d)
                            if best is None or sc < best[0]:
                                best = (sc, it, a, d)
                        _, it, a, d = best
                        pool.remove(it)
                        acc["A"], acc["D"] = a, d
                        order.append(it)
                    return order

                def do_chunk(cc, m, ev):
                    """Gram of anchor tile m vs candidate cols
                    [1024cc, 1024(cc+1))."""
                    acn, aco = divmod(AOFF + 128 * m, 1024)
                    lh = ftr[:, acn, :, aco:aco + 128]
                    gps = psB.tile([128, 1024], F32, tag="g")
                    w = WMAP[m]
                    mh = [h for h in (w, w + 1) if h // 2 == cc]
                    for half in range(2):
                        h = 2 * cc + half
                        c0 = 512 * half
                        # full 256-deep contraction, one DoubleRow matmul
                        nc.tensor.matmul(
                            gps[:, c0:c0 + 512], lh,
                            ftr[:, h // 2, :, 512 * (h % 2):
                                512 * (h % 2) + 512],
                            start=True, stop=(h not in mh), perf_mode=DR)
                        if h in mh:
                            nc.tensor.matmul(
                                gps[:, c0:c0 + 512], negid[:],
                                wm[:, m, 512 * (h - w):512 * (h - w) + 512],
                                start=False, stop=True,
                                skip_group_check=True)
                    if ev:
                        evt = sbB.tile([128, 1024], BF16, tag="ev")
                        nc.scalar.copy(evt[:], gps[:])
                        dummy = sbB.tile([128, 1024], BF16, tag="dum")
                        nc.vector.tensor_scalar(
                            dummy[:], evt[:], 0.0, None, OP.add, OP.max,
                            accum_out=bpmax[:, m, cc:cc + 1])
                        # hardest-pos half-mins ride the same bf16 data:
                        # with BIG=2 the shifted positives sit in [-3,-1]
                        # where bf16 granularity (<=2^-7 rel) is harmless.
                        for h in mh:
                            c0 = 512 * (h % 2)
                            dm2 = sbB.tile([128, 512], BF16, tag="dm2")
                            nc.vector.tensor_scalar(
                                dm2[:], evt[:, c0:c0 + 512], 0.0, None,
                                OP.add, OP.min,
                                accum_out=pmin[:, m, h - w:h - w + 1])
                    else:
                        nc.vector.tensor_reduce(
                            bpmax[:, m, cc:cc + 1], gps[:], axis=AX.X,
                            op=OP.max)

                for cc, m, ev in build_order():
                    do_chunk(cc, m, ev)

            # ---------------- epilogue -----------------------------------
            with ExitStack() as cctx:
                ep = cctx.enter_context(tc.tile_pool(name="ep", bufs=1))

                hh2 = ep.tile([128, 2, MT], F32)
                # pos gram = min of half-mins + 2 ; neg gram = max
                nc.vector.tensor_tensor(
                    hh2[:, 0, :], pmin[:, :, 0], pmin[:, :, 1], op=OP.min)
                nc.vector.tensor_scalar_add(hh2[:, 0, :], hh2[:, 0, :], BIG)
                nc.vector.tensor_reduce(
                    hh2[:, 1, :], bpmax[:], axis=AX.X, op=OP.max)

                # d = sqrt(-2g + 2), affine fused into the ACT sqrt;
                # negative inputs (g marginally > 1) clamp to sqrt(0)=0.
                hhf = hh2[:].rearrange("p a b -> p (a b)")
                y = ep.tile([128, 2 * MT], F32)
                nc.scalar.activation(
                    y[:], hhf, mybir.ActivationFunctionType.Sqrt,
                    bias=bias2[:], scale=-2.0)
                yv = y[:].rearrange("p (a b) -> p a b", a=2)
                loss = ep.tile([128, MT], F32)
                nc.vector.tensor_sub(loss[:], yv[:, 0, :], yv[:, 1, :])
                nc.vector.tensor_scalar(
                    loss[:], loss[:], 0.3, 0.0, OP.add, OP.max)

                # per-partition row sums; the host adds the 128 values
                rowsum = ep.tile([128, 1], F32)
                nc.vector.tensor_reduce(
                    rowsum[:], loss[:], axis=AX.X, op=OP.add)
                nc.sync.dma_start(out_d[:], rowsum[:])

    nc.compile()
    return nc


def _prep_inputs(features, labels):
    feats = np.asarray(features, dtype=np.float32)
    labs = np.asarray(labels)
    order = np.argsort(labs, kind="stable")
    sf = np.ascontiguousarray(feats[order])
    sl = labs[order]
    nrm = np.sqrt((sf.astype(np.float64) ** 2).sum(axis=1, keepdims=True))
    fh = (sf / np.maximum(nrm, 1e-12)).astype(np.float32)
    s_g = np.searchsorted(sl, sl, side="left").astype(np.int64)
    e_g = np.searchsorted(sl, sl, side="right").astype(np.int64)

    jj = np.arange(1024)
    in_maps = []
    for c in range(NCORES):
        off = (CA * c - AOFF) % N
        rot = np.roll(fh, -off, axis=0)          # rot[j] = fh[(j+off) % N]
        ftr = np.ascontiguousarray(
            rot.T.reshape(2, 128, NCH, 1024).transpose(1, 2, 0, 3)
        ).astype(ml_dtypes.float8_e4m3)

        wm = np.zeros((128, MT, 1024), ml_dtypes.float8_e4m3)
        for m in range(MT):
            r0 = CA * c + 128 * m                # sorted rows of this tile
            s = (s_g[r0:r0 + 128] - off) % N     # rotated col bounds
            e = (e_g[r0:r0 + 128] - off - 1) % N + 1
            w0 = 512 * WMAP[m]
            assert (s >= w0).all() and (e <= w0 + 1024).all() \
                and (s < e).all(), f"window containment violated c={c} m={m}"
            wm[:, m, :] = np.where(
                (jj[None, :] >= s[:, None] - w0)
                & (jj[None, :] < e[:, None] - w0), 1.0, 0.0)
        in_maps.append({"ftr": ftr, "wm": wm})
    return in_maps


def kernel(features, labels):
    from concourse.bass_utils import run_bass_kernel_spmd

    if "nc" not in _CACHE:
        _CACHE["nc"] = _build()
    nc = _CACHE["nc"]

    in_maps = _prep_inputs(features, labels)
    res = run_bass_kernel_spmd(nc, in_maps, core_ids=list(range(NCORES)))
    total = np.float64(0.0)
    for c in range(NCORES):
        total += np.float64(res.results[c]["out"].sum(dtype=np.float64))
    return np.float32(total / N)



# revision 34
# speedup vs baseline: 1.3710x; 1.3710x over previous
"""BatchHardTripletLoss on 8 Trainium2 NeuronCores.

Strategy (row-parallel, per the sharding hint):
  - Host: sort rows by label (loss is a mean over anchors -- any permutation
    is loss-invariant), L2-normalize in f32, and hand each core a ROTATED
    copy of the normalized feature matrix, transposed and cast to fp8 e4m3:
    core c's candidate column j holds sorted row (j + 1024c - 512) mod 8192.
    After rotation every core's 1024 anchors sit at fixed columns
    [512, 1536), and each anchor tile m's positive candidates lie inside a
    fixed 1024-wide window at columns [512*WMAP[m], 512*WMAP[m]+1024).
    All per-core differences are data (rotated ftr, window mask), never
    code, so one NEFF runs SPMD on all 8 cores.  (fp8 quantization of the
    normalized features moves the final mean loss by ~7e-4 relative --
    measured, far inside the 2e-2 gate -- while doubling PE throughput.)
  - Math: with normalized features, d2_ij = 2 - 2*g_ij and sqrt/affine are
    monotone, so only per-row min/max of the gram matrix are needed.
    hardest_neg gram = max over all candidates of (g - 2*pos_mask)
    hardest_pos gram = min over window of (g - 2*pos_mask) + 2
    (the -2 penalty is fused into the PE accumulation as an extra matmul
    of a constant -2*I against the fp8 mask -- no NxN elementwise pass;
    g >= -1 for unit vectors, so any penalized positive sits below every
    negative, and the shift keeps values in [-3,1] where bf16 is fine).
  - Device per core: 64 gram chunks (8 anchor tiles x 8 column chunks of
    1024); the 256-deep contraction runs as one fp8 DoubleRow matmul per
    512 output cols.  The two window chunks of each anchor tile get the
    mask fused in PSUM.  PSUM (4 chunk slots) is drained by two engines
    in parallel: ScalarE evicts a chunk to SBUF bf16 and DVE reduces it
    in 4x mode (window chunks also yield the hardest-pos half-mins
    there), or DVE reduces directly from PSUM at 1x; the chunk order and
    evict/direct split are chosen greedily at build time so both drain
    engines stay busy.  Tiny f32 epilogue (sqrt via ACT with the
    d2 = -2g+2 affine fused in, relu, row-sum, partition-sum via matmul
    with ones) produces one scalar per core; host averages.
"""

import numpy as np
import ml_dtypes

N = 8192
D = 256
NCORES = 8
CA = N // NCORES          # anchors per core
MT = CA // 128            # 8 anchor tiles per core
NCH = N // 1024           # 8 candidate DMA chunks of 1024
WMAP = [0, 1, 1, 1, 1, 2, 2, 2]   # window start half (of 512) per anchor tile
BIG = 2.0
AOFF = 512                # anchors occupy rotated cols [512, 1536)

_CACHE = {}


def _build(reps=1):
    import concourse.bass as bass
    import concourse.tile as tile
    from concourse import bacc, mybir, masks
    from contextlib import ExitStack

    F32 = mybir.dt.float32
    BF16 = mybir.dt.bfloat16
    FP8 = mybir.dt.float8e4
    DR = mybir.MatmulPerfMode.DoubleRow
    OP = mybir.AluOpType
    AX = mybir.AxisListType

    nc = bacc.Bacc("TRN2", target_bir_lowering=False, debug=False,
                   num_devices=NCORES)

    ftr_d = nc.dram_tensor("ftr", [128, NCH, 2, 1024], FP8,
                           kind="ExternalInput").ap()
    wm_d = nc.dram_tensor("wm", [128, MT, 1024], FP8,
                          kind="ExternalInput").ap()
    out_d = nc.dram_tensor("out", [128, 1], F32,
                           kind="ExternalOutput").ap()

    with tile.TileContext(nc) as tc:
      with ExitStack() as octx:
        # Constants once; the input tiles double-buffer across reps so
        # rep i+1's DMAs overlap rep i's compute instead of WAR-stalling
        # on the same SBUF addresses.
        constp = octx.enter_context(tc.tile_pool(name="const", bufs=1))
        ident = constp.tile([128, 128], F32)
        masks.make_identity(nc, ident[:])
        # First ACT op is a sqrt so bass loads a table set that holds
        # both sqrt and copy -- avoids a mid-kernel table switch.
        warm = constp.tile([1, 1], F32)
        nc.gpsimd.memset(warm[:], 1.0)
        nc.scalar.sqrt(warm[:], warm[:])
        negid = constp.tile([128, 128], FP8)
        nc.scalar.mul(negid[:], ident[:], -BIG)
        bias2 = constp.tile([128, 1], F32)
        nc.gpsimd.memset(bias2[:], 2.0)
        bigp = octx.enter_context(tc.tile_pool(name="big", bufs=2))
        vecp = octx.enter_context(tc.tile_pool(name="vec", bufs=2))

        for _rep in range(reps):
          with ExitStack() as ctx:
            ftr = bigp.tile([128, NCH, 2, 1024], FP8, tag="ftr")
            wm = bigp.tile([128, MT, 1024], FP8, tag="wm")

            # Two DMA queues (SP + ACT) in parallel; anchor/first-sweep
            # chunks 0-3 land first, mask before the p=0 sweep needs it.
            nc.sync.dma_start(ftr[:, 0], ftr_d[:, 0])
            nc.scalar.dma_start(ftr[:, 1], ftr_d[:, 1])
            nc.sync.dma_start(ftr[:, 2], ftr_d[:, 2])
            nc.scalar.dma_start(ftr[:, 3], ftr_d[:, 3])
            nc.sync.dma_start(ftr[:, 4], ftr_d[:, 4])
            nc.scalar.dma_start(ftr[:, 5], ftr_d[:, 5])
            nc.sync.dma_start(ftr[:, 6], ftr_d[:, 6])
            nc.scalar.dma_start(ftr[:, 7], ftr_d[:, 7])
            # mask last: the window chunks run late in the chunk order
            nc.sync.dma_start(wm[:], wm_d[:])

            bpmax = vecp.tile([128, MT, NCH], F32, tag="bpmax")
            pmin = vecp.tile([128, MT, 2], F32, tag="pmin")

            with ExitStack() as bctx:
                psB = bctx.enter_context(
                    tc.tile_pool(name="psB", bufs=4, space="PSUM"))
                sbB = bctx.enter_context(tc.tile_pool(name="sbB", bufs=6))

                # Chunk visit order + drain-engine assignment: the p=1
                # sweep warms up on chunks 0-3 with strict evict/direct
                # alternation; the rest (window chunks, which are always
                # evicted and also yield the bf16 hardest-pos mins, plus
                # the remaining plain chunks) are ordered greedily so the
                # running ScalarE and DVE drain totals stay balanced.
                def build_order():
                    mins_of = lambda cc, m: sum(
                        1 for h in (WMAP[m], WMAP[m] + 1) if h // 2 == cc)
                    order = []
                    for i, m in enumerate((0, 3, 2, 5, 4, 7, 6, 1)):
                        order.append((2, m, i % 2 == 0))
                        order.append((3, m, i % 2 == 1))
                    rest = ([(cc, m, True) for cc in (0, 1)
                             for m in range(MT)]
                            + [(cc, m, True) for cc in (4, 6)
                               for m in range(MT)]
                            + [(cc, m, True) for cc in (5,)
                               for m in (0,)]
                            + [(cc, m, False) for cc in (5,)
                               for m in (1, 2, 3, 4, 5, 6, 7)]
                            + [(cc, m, False) for cc in (7,)
                               for m in range(MT)])
                    acc = {"A": 0.0, "D": 0.0}
                    pool = list(rest)
                    while pool:
                        # pick the candidate that keeps |A - D| smallest;
                        # window chunks wait until slot 12 (mask DMA).
                        best = None
                        for it in pool:
                            cc, m, ev = it
                            if cc < 2 and len(order) < 28:
                                continue
                            a, d = acc["A"], acc["D"]
                            if ev:
                                a += 1038
                                d += 327 + 194 * mins_of(cc, m)
                            else:
                                d += 1127
                            sc = abs(a - d)
                            if best is None or sc < best[0]:
                                best = (sc, it, a, d)
                        _, it, a, d = best
                        pool.remove(it)
                        acc["A"], acc["D"] = a, d
                        order.append(it)
                    return order

                def do_chunk(cc, m, ev):
                    """Gram of anchor tile m vs candidate cols
                    [1024cc, 1024(cc+1))."""
                    acn, aco = divmod(AOFF + 128 * m, 1024)
                    lh = ftr[:, acn, :, aco:aco + 128]
                    gps = psB.tile([128, 1024], F32, tag="g")
                    w = WMAP[m]
                    mh = [h for h in (w, w + 1) if h // 2 == cc]
                    for half in range(2):
                        h = 2 * cc + half
                        c0 = 512 * half
                        # full 256-deep contraction, one DoubleRow matmul
                        nc.tensor.matmul(
                            gps[:, c0:c0 + 512], lh,
                            ftr[:, h // 2, :, 512 * (h % 2):
                                512 * (h % 2) + 512],
                            start=True, stop=(h not in mh), perf_mode=DR)
                        if h in mh:
                            nc.tensor.matmul(
                                gps[:, c0:c0 + 512], negid[:],
                                wm[:, m, 512 * (h - w):512 * (h - w) + 512],
                                start=False, stop=True,
                                skip_group_check=True)
                    if ev:
                        evt = sbB.tile([128, 1024], BF16, tag="ev")
                        nc.scalar.copy(evt[:], gps[:])
                        dummy = sbB.tile([128, 1024], BF16, tag="dum")
                        nc.vector.tensor_scalar(
                            dummy[:], evt[:], 0.0, None, OP.add, OP.max,
                            accum_out=bpmax[:, m, cc:cc + 1])
                        # hardest-pos half-mins ride the same bf16 data:
                        # with BIG=2 the shifted positives sit in [-3,-1]
                        # where bf16 granularity (<=2^-7 rel) is harmless.
                        for h in mh:
                            c0 = 512 * (h % 2)
                            dm2 = sbB.tile([128, 512], BF16, tag="dm2")
                            nc.vector.tensor_scalar(
                                dm2[:], evt[:, c0:c0 + 512], 0.0, None,
                                OP.add, OP.min,
                                accum_out=pmin[:, m, h - w:h - w + 1])
                    else:
                        nc.vector.tensor_reduce(
                            bpmax[:, m, cc:cc + 1], gps[:], axis=AX.X,
                            op=OP.max)

                for cc, m, ev in build_order():
                    do_chunk(cc, m, ev)

            # ---------------- epilogue -----------------------------------
            with ExitStack() as cctx:
                ep = cctx.enter_context(tc.tile_pool(name="ep", bufs=1))

                hh2 = ep.tile([128, 2, MT], F32)
                # pos gram = min of half-mins + 2 ; neg gram = max
                nc.vector.tensor_tensor(
                    hh2[:, 0, :], pmin[:, :, 0], pmin[:, :, 1], op=OP.min)
                nc.vector.tensor_scalar_add(hh2[:, 0, :], hh2[:, 0, :], BIG)
                nc.vector.tensor_reduce(
                    hh2[:, 1, :], bpmax[:], axis=AX.X, op=OP.max)

                # d = sqrt(-2g + 2), affine fused into the ACT sqrt;
                # negative inputs (g marginally > 1) clamp to sqrt(0)=0.
                hhf = hh2[:].rearrange("p a b -> p (a b)")
                y = ep.tile([128, 2 * MT], F32)
                nc.scalar.activation(
                    y[:], hhf, mybir.ActivationFunctionType.Sqrt,
                    bias=bias2[:], scale=-2.0)
                yv = y[:].rearrange("p (a b) -> p a b", a=2)
                loss = ep.tile([128, MT], F32)
                nc.vector.tensor_sub(loss[:], yv[:, 0, :], yv[:, 1, :])
                nc.vector.tensor_scalar(
                    loss[:], loss[:], 0.3, 0.0, OP.add, OP.max)

                # per-partition row sums; the host adds the 128 values
                rowsum = ep.tile([128, 1], F32)
                nc.vector.tensor_reduce(
                    rowsum[:], loss[:], axis=AX.X, op=OP.add)
                nc.sync.dma_start(out_d[:], rowsum[:])

    nc.compile()
    return nc


def _prep_inputs(features, labels):
    feats = np.asarray(features, dtype=np.float32)
    labs = np.asarray(labels)
    order = np.argsort(labs, kind="stable")
    sf = np.ascontiguousarray(feats[order])
    sl = labs[order]
    nrm = np.sqrt((sf.astype(np.float64) ** 2).sum(axis=1, keepdims=True))
    fh = (sf / np.maximum(nrm, 1e-12)).astype(np.float32)
    s_g = np.searchsorted(sl, sl, side="left").astype(np.int64)
    e_g = np.searchsorted(sl, sl, side="right").astype(np.int64)

    jj = np.arange(1024)
    in_maps = []
    for c in range(NCORES):
        off = (CA * c - AOFF) % N
        rot = np.roll(fh, -off, axis=0)          # rot[j] = fh[(j+off) % N]
        ftr = np.ascontiguousarray(
            rot.T.reshape(2, 128, NCH, 1024).transpose(1, 2, 0, 3)
        ).astype(ml_dtypes.float8_e4m3)

        wm = np.zeros((128, MT, 1024), ml_dtypes.float8_e4m3)
        for m in range(MT):
            r0 = CA * c + 128 * m                # sorted rows of this tile
            s = (s_g[r0:r0 + 128] - off) % N     # rotated col bounds
            e = (e_g[r0:r0 + 128] - off - 1) % N + 1
            w0 = 512 * WMAP[m]
            assert (s >= w0).all() and (e <= w0 + 1024).all() \
                and (s < e).all(), f"window containment violated c={c} m={m}"
            wm[:, m, :] = np.where(
                (jj[None, :] >= s[:, None] - w0)
                & (jj[None, :] < e[:, None] - w0), 1.0, 0.0)
        in_maps.append({"ftr": ftr, "wm": wm})
    return in_maps


def kernel(features, labels):
    from concourse.bass_utils import run_bass_kernel_spmd

    if "nc" not in _CACHE:
        _CACHE["nc"] = _build()
    nc = _CACHE["nc"]

    in_maps = _prep_inputs(features, labels)
    res = run_bass_kernel_spmd(nc, in_maps, core_ids=list(range(NCORES)))
    total = np.float64(0.0)
    for c in range(NCORES):
        total += np.float64(res.results[c]["out"].sum(dtype=np.float64))
    return np.float32(total / N)

